# revision 15
# baseline (speedup 1.0000x reference)
"""Trainium2 Bass kernel for an ActorCritic module (JSSP-style).

Math (per graph b of 64):
  h       = relu(x @ fe_w1 + fe_b1)                  x:[10000,3] -> h:[10000,64]
  h_nodes = relu(h @ fe_w2 + fe_b2)                  -> [10000,128]
  pooled  = [mean_n h_nodes, max_n h_nodes]          -> [256]
  cand    = h_nodes[candidate]                       -> [201,128]
  scores  = tanh-MLP([cand, pooled]) -> softmax(masked) -> pi [201,1]
  v       = tanh-MLP(pooled)                         -> [1]

Sharding: pure data-parallel over batch, 8 graphs per NeuronCore, weights
replicated. No collectives.

On-device layout: everything feature-on-partitions. x is packed two nodes per
column ([6,5000] per graph) so the layer-1 matmul uses a block-diagonal weight
with M=128. Layer 2 runs as two concurrent row-group matmuls (even nodes on
partitions 0-63, odd on 64-127). PSUM eviction fuses bias+relu+cast-to-bf16
and the mean-pool accumulation (ScalarE activation accum / DVE tensor_scalar
accum). Max-pool is a pairwise
TT-max tree split between GPSIMD and DVE. Candidate gather via GPSIMD
ap_gather. Actor/critic MLPs are small batched matmuls; softmax runs per
2-graph chunk.
"""

import os
from contextlib import ExitStack

import numpy as np

import concourse.bass as bass
from concourse import bacc
import concourse.mybir as mybir
import concourse.tile as tile
from concourse.bass_utils import run_bass_kernel_spmd

# ---------------------------------------------------------------- constants
B, N_NODES, IN_DIM = 64, 10000, 3
N_CAND = 201
HID, ND, HA = 64, 128, 256       # hidden, node dim, actor/critic hidden
NCORES = 8
G = B // NCORES                  # graphs per core = 8
NH = N_NODES // 2                # packed columns per graph = 5000
CAND = 208                       # candidates padded to a multiple of 16
ACT_N = G * CAND                 # actor batch columns = 1664
CHUNK = 2 * CAND                 # actor n-chunk (2 graphs) = 416
BIG_NEG = -30000.0

F32 = mybir.dt.float32
F16 = mybir.dt.float16
U16 = mybir.dt.uint16

ADD = mybir.AluOpType.add
MAX = mybir.AluOpType.max
MULT = mybir.AluOpType.mult
RELU = mybir.ActivationFunctionType.Relu
TANH = mybir.ActivationFunctionType.Tanh
EXP = mybir.ActivationFunctionType.Exp
AXX = mybir.AxisListType.X


def _col_tiles(total, step=512):
    tiles, off = [], 0
    while off < total:
        n = min(step, total - off)
        tiles.append((off, n))
        off += n
    return tiles


L1_TILES = _col_tiles(NH)        # 10 tiles: 9x512 + 392

# Which eviction columns run on DVE (rest on ScalarE). Tuned for engine balance.
L1_DVE = {5}
L2_DVE = {6, 13, 19}


# ---------------------------------------------------------------- builder
def _build_bass():
    nc = bacc.Bacc("TRN2", debug=False)

    xp = nc.declare_dram_parameter("xp", [G, 6, NH], F16, isOutput=False)
    w1 = nc.declare_dram_parameter("w1", [6, 128], F16, isOutput=False)
    b1 = nc.declare_dram_parameter("b1", [128, 1], F32, isOutput=False)
    w2 = nc.declare_dram_parameter("w2", [128, 128], F16, isOutput=False)
    b2 = nc.declare_dram_parameter("b2", [128, 1], F32, isOutput=False)
    aw1 = nc.declare_dram_parameter("aw1", [384, 256], F16, isOutput=False)
    ab1 = nc.declare_dram_parameter("ab1", [128, 2], F32, isOutput=False)
    aw2 = nc.declare_dram_parameter("aw2", [256, 256], F16, isOutput=False)
    ab2 = nc.declare_dram_parameter("ab2", [128, 2], F32, isOutput=False)
    aw3 = nc.declare_dram_parameter("aw3", [128, 2], F16, isOutput=False)
    cw1 = nc.declare_dram_parameter("cw1", [256, 256], F32, isOutput=False)
    cb1 = nc.declare_dram_parameter("cb1", [128, 2], F32, isOutput=False)
    cw2 = nc.declare_dram_parameter("cw2", [256, 256], F32, isOutput=False)
    cb2 = nc.declare_dram_parameter("cb2", [128, 2], F32, isOutput=False)
    cw3 = nc.declare_dram_parameter("cw3", [128, 2], F32, isOutput=False)
    cb3 = nc.declare_dram_parameter("cb3", [1, 1], F32, isOutput=False)
    cidx = nc.declare_dram_parameter("cidx", [G, 128, CAND // 16], U16, isOutput=False)
    mskp = nc.declare_dram_parameter("mskp", [1, ACT_N], F32, isOutput=False)
    out_pi = nc.declare_dram_parameter("out_pi", [1, ACT_N], F32, isOutput=True)
    out_v = nc.declare_dram_parameter("out_v", [1, G], F32, isOutput=True)
    out_dbg = nc.declare_dram_parameter("out_dbg", [128, 2 * G], F32, isOutput=True)

    with tile.TileContext(nc) as tc, ExitStack() as ctx:
        const = ctx.enter_context(tc.tile_pool(name="const", bufs=1))
        xpool = ctx.enter_context(tc.tile_pool(name="xp", bufs=2))
        hpool = ctx.enter_context(tc.tile_pool(name="hp", bufs=2))
        rpool = ctx.enter_context(tc.tile_pool(name="rp", bufs=2))
        gpool = ctx.enter_context(tc.tile_pool(name="gp", bufs=2))
        tpool = ctx.enter_context(tc.tile_pool(name="tp", bufs=2))
        ps1 = ctx.enter_context(tc.tile_pool(name="ps1", bufs=2, space="PSUM"))
        ps2 = ctx.enter_context(tc.tile_pool(name="ps2", bufs=4, space="PSUM"))

        # ---- load weights / constants
        def load(shape, dtype, src, name):
            t = const.tile(shape, dtype, tag=name)
            nc.sync.dma_start(t[:], src)
            return t

        w1_s = load([6, 128], F16, w1[:], "w1")
        b1_s = load([128, 1], F32, b1[:], "b1")
        w2_s = load([128, 128], F16, w2[:], "w2")
        b2_s = load([128, 1], F32, b2[:], "b2")
        aw1_s = load([128, 3, 256], F16, aw1[:].rearrange("(k p) m -> p k m", p=128), "aw1")
        ab1_s = load([128, 2], F32, ab1[:], "ab1")
        aw2_s = load([128, 2, 256], F16, aw2[:].rearrange("(k p) m -> p k m", p=128), "aw2")
        ab2_s = load([128, 2], F32, ab2[:], "ab2")
        aw3_s = load([128, 2], F16, aw3[:], "aw3")
        cw1_s = load([128, 2, 256], F32, cw1[:].rearrange("(k p) m -> p k m", p=128), "cw1")
        cb1_s = load([128, 2], F32, cb1[:], "cb1")
        cw2_s = load([128, 2, 256], F32, cw2[:].rearrange("(k p) m -> p k m", p=128), "cw2")
        cb2_s = load([128, 2], F32, cb2[:], "cb2")
        cw3_s = load([128, 2], F32, cw3[:], "cw3")
        cb3_s = load([1, 1], F32, cb3[:], "cb3")
        msk_s = load([1, ACT_N], F32, mskp[:], "msk")

        candT = const.tile([128, ACT_N], F16, tag="candT")
        sums = const.tile([128, G], F32, tag="sums")
        maxs = const.tile([128, G], F32, tag="maxs")

        # ---- per-graph main loop
        for g in range(G):
            xg = xpool.tile([6, NH], F16, tag="x")
            nc.sync.dma_start(xg[:], xp[g])
            idxg = gpool.tile([128, CAND // 16], U16, tag="idx")
            nc.sync.dma_start(idxg[:], cidx[g])

            # L1: hT packed [128, 5000] = relu(blockdiag(W1)^T @ x_packed + b1bd)
            hg = hpool.tile([128, NH], F16, tag="h")
            for t, (off, n) in enumerate(L1_TILES):
                ps = ps1.tile([128, 512], F32, tag="ps1")
                nc.tensor.matmul(
                    ps[:, :n],
                    w1_s[:],
                    xg[:, off:off + n],
                )
                dst = hg[:, off:off + n]
                if t in L1_DVE:
                    nc.vector.tensor_scalar(
                        out=dst, in0=ps[:, :n],
                        scalar1=b1_s[:, 0:1], scalar2=0.0, op0=ADD, op1=MAX,
                    )
                else:
                    nc.scalar.activation(dst, ps[:, :n], RELU, bias=b1_s[:, 0:1])

            # L2: z = W2^T @ h per row-group; fused bias+relu+sum eviction
            rg = rpool.tile([128, 2 * NH], F16, tag="r")
            gsum = gpool.tile([128, 20], F32, tag="gsum")
            for s in range(2):
                p0 = 64 * s
                for t, (off, n) in enumerate(L1_TILES):
                    col = s * 10 + t
                    ps = ps2.tile([128, 512], F32, tag="ps2")
                    nc.tensor.matmul(
                        ps[:, :n],
                        w2_s[p0:p0 + 64, :],
                        hg[p0:p0 + 64, off:off + n],
                    )
                    dst = rg[:, s * NH + off: s * NH + off + n]
                    if col in L2_DVE:
                        nc.vector.tensor_scalar(
                            out=dst, in0=ps[:, :n],
                            scalar1=b2_s[:, 0:1], scalar2=0.0, op0=ADD, op1=MAX,
                            accum_out=gsum[:, col:col + 1],
                        )
                    else:
                        nc.scalar.activation(
                            dst, ps[:, :n], RELU, bias=b2_s[:, 0:1],
                            accum_out=gsum[:, col:col + 1],
                        )

            # candidate gather (must read rg before the in-place max folds)
            nc.gpsimd.indirect_copy(
                candT[:, g * CAND:(g + 1) * CAND],
                rg[:],
                idxg[:],
                i_know_ap_gather_is_preferred=True,
            )

            # mean-pool: total per-graph sum (scaled later)
            nc.vector.tensor_reduce(sums[:, g:g + 1], gsum[:], axis=AXX, op=ADD)

            # max-pool: pairwise TT-max tree on DVE (bf16 2x mode)
            da = gpool.tile([128, 2500], F16, tag="da")
            db = gpool.tile([128, 2500], F16, tag="db")
            nc.vector.tensor_tensor(da[:], rg[:, 0:2500], rg[:, 2500:5000], op=MAX)
            nc.vector.tensor_tensor(db[:], rg[:, 5000:7500], rg[:, 7500:10000], op=MAX)
            nc.vector.tensor_tensor(da[:], da[:], db[:], op=MAX)
            nc.vector.tensor_tensor(da[:, 0:1250], da[:, 0:1250], da[:, 1250:2500], op=MAX)
            nc.vector.tensor_tensor(da[:, 0:625], da[:, 0:625], da[:, 625:1250], op=MAX)
            nc.vector.tensor_reduce(maxs[:, g:g + 1], da[:, 0:625], axis=AXX, op=MAX)

        # ---- pooled features: fp16 [mean | max] for the actor, fp32 for the critic
        mm = const.tile([128, 2 * G], F16, tag="mm")
        nc.vector.tensor_scalar(out=mm[:, 0:G], in0=sums[:], scalar1=1.0 / N_NODES,
                                scalar2=None, op0=MULT)
        nc.vector.tensor_copy(mm[:, G:2 * G], maxs[:])
        meanf = const.tile([128, G], F32, tag="meanf")
        nc.vector.tensor_scalar(out=meanf[:], in0=sums[:], scalar1=1.0 / N_NODES,
                                scalar2=None, op0=MULT)

        # ---- actor pooled contribution: pp[m] = aw1[128:256,m]^T mean + aw1[256:384,m]^T max
        pp = const.tile([128, 2, G], F32, tag="pp")
        for m in range(2):
            ps = ps1.tile([128, 512], F32, tag="ps1")
            nc.tensor.matmul(ps[:, :G], aw1_s[:, 1, m * 128:(m + 1) * 128],
                             mm[:, 0:G], start=True, stop=False)
            nc.tensor.matmul(ps[:, :G], aw1_s[:, 2, m * 128:(m + 1) * 128],
                             mm[:, G:2 * G], start=False, stop=True)
            nc.vector.tensor_copy(pp[:, m, :], ps[:, :G])

        # ---- critic (tiny): v = cw3^T tanh(cw2^T tanh(cw1^T pooled + cb1) + cb2) + cb3
        c1 = const.tile([128, 2, G], F32, tag="c1")
        for m in range(2):
            ps = ps2.tile([128, 512], F32, tag="ps2")
            nc.tensor.matmul(ps[:, :G], cw1_s[:, 0, m * 128:(m + 1) * 128],
                             meanf[:], start=True, stop=False)
            nc.tensor.matmul(ps[:, :G], cw1_s[:, 1, m * 128:(m + 1) * 128],
                             maxs[:], start=False, stop=True)
            nc.scalar.activation(c1[:, m, :], ps[:, :G], TANH, bias=cb1_s[:, m:m + 1])
        c2 = const.tile([128, 2, G], F32, tag="c2")
        for m in range(2):
            ps = ps2.tile([128, 512], F32, tag="ps2")
            nc.tensor.matmul(ps[:, :G], cw2_s[:, 0, m * 128:(m + 1) * 128],
                             c1[:, 0, :], start=True, stop=False)
            nc.tensor.matmul(ps[:, :G], cw2_s[:, 1, m * 128:(m + 1) * 128],
                             c1[:, 1, :], start=False, stop=True)
            nc.scalar.activation(c2[:, m, :], ps[:, :G], TANH, bias=cb2_s[:, m:m + 1])
        vps = ps2.tile([128, 512], F32, tag="ps2")
        nc.tensor.matmul(vps[0:1, :G], cw3_s[:, 0:1], c2[:, 0, :],
                         start=True, stop=False)
        nc.tensor.matmul(vps[0:1, :G], cw3_s[:, 1:2], c2[:, 1, :],
                         start=False, stop=True)
        dbg = const.tile([128, 2 * G], F32, tag="dbg")
        nc.vector.tensor_copy(dbg[:, 0:G], meanf[:])
        nc.vector.tensor_copy(dbg[:, G:2 * G], maxs[:])
        nc.sync.dma_start(out_dbg[:], dbg[:])
        vsb = tpool.tile([1, G], F32, tag="v")
        nc.vector.tensor_scalar(out=vsb[:], in0=vps[0:1, :G],
                                scalar1=cb3_s[0:1, 0:1], scalar2=None, op0=ADD)
        nc.sync.dma_start(out_v[:], vsb[:])

        # ---- actor MLP over 4 chunks of 2 graphs (416 cols)
        a1 = const.tile([128, 2, ACT_N], F16, tag="a1")
        a2 = const.tile([128, 2, ACT_N], F16, tag="a2")
        for c in range(4):
            lo, hi = c * CHUNK, (c + 1) * CHUNK
            for m in range(2):
                ps = ps1.tile([128, 512], F32, tag="ps1")
                nc.tensor.matmul(ps[:, :CHUNK],
                                 aw1_s[:, 0, m * 128:(m + 1) * 128], candT[:, lo:hi])
                tmp = tpool.tile([128, CHUNK], F32, tag="a1tmp")
                ppb = pp[:, m, 2 * c:2 * c + 2].unsqueeze(2).broadcast_to([128, 2, CAND])
                nc.vector.tensor_tensor(tmp[:], ps[:, :CHUNK], ppb, op=ADD)
                nc.scalar.activation(a1[:, m, lo:hi], tmp[:], TANH, bias=ab1_s[:, m:m + 1])
            for m in range(2):
                ps = ps1.tile([128, 512], F32, tag="ps1")
                nc.tensor.matmul(ps[:, :CHUNK],
                                 aw2_s[:, 0, m * 128:(m + 1) * 128], a1[:, 0, lo:hi],
                                 start=True, stop=False)
                nc.tensor.matmul(ps[:, :CHUNK],
                                 aw2_s[:, 1, m * 128:(m + 1) * 128], a1[:, 1, lo:hi],
                                 start=False, stop=True)
                nc.scalar.activation(a2[:, m, lo:hi], ps[:, :CHUNK], TANH,
                                     bias=ab2_s[:, m:m + 1])
            sps = ps2.tile([128, 512], F32, tag="ps2")
            nc.tensor.matmul(sps[0:1, :CHUNK], aw3_s[:, 0:1], a2[:, 0, lo:hi],
                             start=True, stop=False)
            nc.tensor.matmul(sps[0:1, :CHUNK], aw3_s[:, 1:2], a2[:, 1, lo:hi],
                             start=False, stop=True)
            # masked softmax per graph segment
            sm = tpool.tile([1, CHUNK], F32, tag="sm")
            nc.vector.tensor_tensor(sm[:], sps[0:1, :CHUNK], msk_s[0:1, lo:hi], op=ADD)
            es = tpool.tile([1, CHUNK], F32, tag="es")
            nc.scalar.activation(es[:], sm[:], EXP)
            ssum = tpool.tile([1, 2], F32, tag="ssum")
            nc.vector.tensor_reduce(ssum[:], es[:].rearrange("p (g c) -> p g c", g=2),
                                    axis=AXX, op=ADD)
            rec = tpool.tile([1, 2], F32, tag="rec")
            nc.vector.reciprocal(rec[:], ssum[:])
            pi = tpool.tile([1, CHUNK], F32, tag="pi")
            recb = rec[:].unsqueeze(2).broadcast_to([1, 2, CAND])
            nc.vector.tensor_tensor(pi[:].rearrange("p (g c) -> p g c", g=2),
                                    es[:].rearrange("p (g c) -> p g c", g=2),
                                    recb, op=MULT)
            nc.sync.dma_start(out_pi[0:1, lo:hi], pi[:])

    nc.compile()
    return nc


# ---------------------------------------------------------------- host packing
def _f16(a):
    return np.asarray(a, np.float32).astype(np.float16)


def _pack_shared(i):
    """Weights / biases shared by all cores."""
    fe_w1 = np.asarray(i["fe_w1"], np.float32)
    fe_b1 = np.asarray(i["fe_b1"], np.float32)
    fe_w2 = np.asarray(i["fe_w2"], np.float32)
    fe_b2 = np.asarray(i["fe_b2"], np.float32)

    w1bd = np.zeros((6, 128), np.float32)
    w1bd[0:3, 0:64] = fe_w1
    w1bd[3:6, 64:128] = fe_w1
    b1bd = np.concatenate([fe_b1, fe_b1]).reshape(128, 1)
    w2ab = np.concatenate([fe_w2, fe_w2], axis=0)          # [128,128]
    b2d = fe_b2.reshape(128, 1)

    def cols2(v):                                          # [256] -> [128, 2]
        return np.asarray(v, np.float32).reshape(2, 128).T.copy()

    def k2(w):                                             # [256,1] -> [128, 2]
        return np.asarray(w, np.float32).reshape(2, 128).T.copy()

    return {
        "w1": _f16(w1bd), "b1": b1bd, "w2": _f16(w2ab), "b2": b2d,
        "aw1": _f16(i["a_w1"]), "ab1": cols2(i["a_b1"]),
        "aw2": _f16(i["a_w2"]), "ab2": cols2(i["a_b2"]),
        "aw3": _f16(k2(i["a_w3"])),
        "cw1": np.asarray(i["c_w1"], np.float32), "cb1": cols2(i["c_b1"]),
        "cw2": np.asarray(i["c_w2"], np.float32), "cb2": cols2(i["c_b2"]),
        "cw3": k2(i["c_w3"]),
        "cb3": np.asarray(i["c_b3"], np.float32).reshape(1, 1),
    }


def _pack_core(x, candidate, mask, core):
    gs = slice(core * G, (core + 1) * G)
    xg = np.asarray(x[gs], np.float32)                     # [8,10000,3]
    # packed: xp[g, r, c] = x[g, 2c + r//3, r%3]
    xp = np.ascontiguousarray(
        xg.reshape(G, NH, 2, 3).transpose(0, 2, 3, 1).reshape(G, 6, NH)
        .astype(np.float16))

    cand = np.asarray(candidate[gs]).astype(np.int64)      # [8,201]
    cand_r = (cand % 2) * NH + cand // 2                   # packed column index
    cp = np.zeros((G, CAND), np.int64)
    cp[:, :N_CAND] = cand_r
    # wrap for ap_gather: index j lives at [16*grp + j%16, j//16] for all groups
    idxw = np.zeros((G, 16, CAND // 16), np.uint16)
    j = np.arange(CAND)
    idxw[:, j % 16, j // 16] = cp.astype(np.uint16)
    cidx = np.ascontiguousarray(np.tile(idxw, (1, 8, 1)))  # [8,128,13]

    m = np.asarray(mask[gs]).astype(bool)                  # [8,201]
    mp = np.full((G, CAND), BIG_NEG, np.float32)
    mp[:, :N_CAND] = np.where(m, 0.0, BIG_NEG)
    mskp = np.ascontiguousarray(mp.reshape(1, ACT_N))

    return {"xp": xp, "cidx": cidx, "mskp": mskp}


_CACHE = {}


def _get_nc():
    if "nc" not in _CACHE:
        _CACHE["nc"] = _build_bass()
    return _CACHE["nc"]


def _run(inputs, trace=False):
    nc = _get_nc()
    shared = _pack_shared(inputs)
    x, candidate, mask = inputs["x"], inputs["candidate"], inputs["mask"]
    in_maps = []
    for core in range(NCORES):
        m = dict(shared)
        m.update(_pack_core(x, candidate, mask, core))
        in_maps.append(m)
    res = run_bass_kernel_spmd(nc, in_maps, core_ids=list(range(NCORES)),
                               trace=trace)
    pi = np.concatenate(
        [r["out_pi"].reshape(G, CAND)[:, :N_CAND] for r in res.results], axis=0)
    v = np.concatenate([r["out_v"].reshape(G) for r in res.results], axis=0)
    return (pi.reshape(B, N_CAND, 1).astype(np.float32),
            v.reshape(B, 1).astype(np.float32)), res


def kernel(**inputs):
    out, _ = _run(inputs, trace=False)
    return out


# revision 31
# speedup vs baseline: 1.2942x; 1.2942x over previous
"""Trainium2 Bass kernel for an ActorCritic module (JSSP-style).

Math (per graph b of 64):
  h       = relu(x @ fe_w1 + fe_b1)                  x:[10000,3] -> h:[10000,64]
  h_nodes = relu(h @ fe_w2 + fe_b2)                  -> [10000,128]
  pooled  = [mean_n h_nodes, max_n h_nodes]          -> [256]
  cand    = h_nodes[candidate]                       -> [201,128]
  scores  = tanh-MLP([cand, pooled]) -> softmax(masked) -> pi [201,1]
  v       = tanh-MLP(pooled)                         -> [1]

Sharding: pure data-parallel over batch, 8 graphs per NeuronCore, all weights
replicated, no collectives. Outputs gathered on the host.

Device design (per core, feature-on-partition layout, fp16 compute with fp32
PSUM accumulation; critic in fp32):
 - x is host-packed two-nodes-per-column and quartered so layer 1 runs as four
   concurrent row-tiled matmuls (tile_position row groups, K=6 each, M=128).
 - Layer 2 runs as two concurrent row-group matmuls (even nodes' features on
   partitions 0-63, odd on 64-127), interleaved into [128,2048] PSUM chunks.
 - PSUM eviction fuses bias + relu + fp16 cast + the mean-pool accumulation
   (ScalarE activation accum_out; DVE scalar_tensor_tensor accum for the
   chunks assigned to DVE for engine balance).
 - Max-pool is a pairwise tensor_tensor max tree on DVE (fp16 2x mode).
 - Candidate gather via GPSIMD indirect_copy with host-precomputed wrapped
   uint16 column indices (the node->column remap is done on the host).
 - Actor MLP batched over 2-graph chunks; softmax uses exp without max
   subtraction (scores are tiny; masked entries get -30000 -> exp == 0).
 - All weights/biases ship as two pre-packed SBUF-layout blobs (one f32, one
   f16) so startup is three DMAs instead of seventeen.

Measured on trn2 (8 cores): ~164 us exec, pi relerr ~5e-7, v relerr ~1.3e-3.
"""

import os
from contextlib import ExitStack

import numpy as np

import concourse.bass as bass
from concourse import bacc
import concourse.mybir as mybir
import concourse.tile as tile
from concourse.bass_utils import run_bass_kernel_spmd

# ---------------------------------------------------------------- constants
B, N_NODES, IN_DIM = 64, 10000, 3
N_CAND = 201
HID, ND, HA = 64, 128, 256       # hidden, node dim, actor/critic hidden
NCORES = 8
G = B // NCORES                  # graphs per core = 8
NH = N_NODES // 2                # packed columns per graph = 5000
CAND = 208                       # candidates padded to a multiple of 16
ACT_N = G * CAND                 # actor batch columns = 1664
CHUNK = 2 * CAND                 # actor n-chunk (2 graphs) = 416
BIG_NEG = -30000.0

F32 = mybir.dt.float32
F16 = mybir.dt.float16
U16 = mybir.dt.uint16

ADD = mybir.AluOpType.add
MAX = mybir.AluOpType.max
MULT = mybir.AluOpType.mult
RELU = mybir.ActivationFunctionType.Relu
TANH = mybir.ActivationFunctionType.Tanh
EXP = mybir.ActivationFunctionType.Exp
AXX = mybir.AxisListType.X


def _col_tiles(total, step=512):
    tiles, off = [], 0
    while off < total:
        n = min(step, total - off)
        tiles.append((off, n))
        off += n
    return tiles


L2_NT = _col_tiles(NH)           # 10 n-tiles per stream: 9x512 + 392

# L1 is 4-way row-tiled: quarter q covers nodes [2500q, 2500q+2500), packed
# two nodes per column (1250 quarter-columns). PSUM fill walls per graph:
L1_WALLS = [(0, 512), (512, 512), (1024, 226)]     # (quarter-col offset, n)
L1_CHUNK_BASE = [0, 2048, 4096]                    # h-col base per wall chunk


def _h_col(q, j):
    # h column of quarter q, quarter-column j under the wall layout
    if j < 512:
        return 512 * q + j
    if j < 1024:
        return 2048 + 512 * q + (j - 512)
    return 4096 + 226 * q + (j - 1024)


def _r_col_base(s, m):
    # r column base of L2 stream s (0=even node, 1=odd), n-tile m (0..9)
    t, k = divmod(m, 2)
    sizes = [L2_NT[2 * t][1], L2_NT[2 * t][1],
             L2_NT[2 * t + 1][1], L2_NT[2 * t + 1][1]]
    idx = 2 * k + s
    return 2048 * t + sum(sizes[:idx])


# Which eviction chunks run on DVE (rest on ScalarE). Tuned for engine balance.
L1_DVE = {1}
L2_DVE = {2}


# ---------------------------------------------------------------- builder
DVE_W = 512        # columns of each clean 2048-chunk evicted by DVE (rest ACT)


def _build_bass():
    nc = bacc.Bacc("TRN2", debug=False)

    xp = nc.declare_dram_parameter("xp", [G, 128, NH // 4], F16, isOutput=False)
    # fp32 blob cols: b1(1) b2(1) ab1(2) ab2(2) cb1(2) cb2(2) cb3(1) cw1(512) cw2(512) cw3(2)
    blob32 = nc.declare_dram_parameter("blob32", [128, 1037], F32, isOutput=False)
    # fp16 blob cols: w1(128) w2(128) aw1(768) aw2(512) aw3(2)
    blob16 = nc.declare_dram_parameter("blob16", [128, 1538], F16, isOutput=False)
    cidx = nc.declare_dram_parameter("cidx", [G, 128, CAND // 16], U16, isOutput=False)
    mskp = nc.declare_dram_parameter("mskp", [1, ACT_N], F32, isOutput=False)
    out_pi = nc.declare_dram_parameter("out_pi", [1, ACT_N], F32, isOutput=True)
    out_v = nc.declare_dram_parameter("out_v", [1, G], F32, isOutput=True)

    with tile.TileContext(nc) as tc, ExitStack() as ctx:
        const = ctx.enter_context(tc.tile_pool(name="const", bufs=1))
        xpool = ctx.enter_context(tc.tile_pool(name="xp", bufs=3))
        hpool = ctx.enter_context(tc.tile_pool(name="hp", bufs=2))
        rpool = ctx.enter_context(tc.tile_pool(name="rp", bufs=3))
        gpool = ctx.enter_context(tc.tile_pool(name="gp", bufs=3))
        tpool = ctx.enter_context(tc.tile_pool(name="tp", bufs=2))
        pspool = ctx.enter_context(tc.tile_pool(name="ps", bufs=2, space="PSUM"))

        # ---- load weights / constants
        def load(shape, dtype, src, name):
            t = const.tile(shape, dtype, tag=name)
            nc.sync.dma_start(t[:], src)
            return t

        bl32 = load([128, 1037], F32, blob32[:], "bl32")
        bl16 = load([128, 1538], F16, blob16[:], "bl16")
        msk_s = load([1, ACT_N], F32, mskp[:], "msk")
        b1_s = bl32[:, 0:1]
        b2_s = bl32[:, 1:2]
        ab1_s = bl32[:, 2:4]
        ab2_s = bl32[:, 4:6]
        cb1_s = bl32[:, 6:8]
        cb2_s = bl32[:, 8:10]
        cb3_s = bl32[:, 10:11]
        cw1_s = bl32[:, 11:523].rearrange("p (k m) -> p k m", m=256)
        cw2_s = bl32[:, 523:1035].rearrange("p (k m) -> p k m", m=256)
        cw3_s = bl32[:, 1035:1037]
        w1_s = bl16[:, 0:128]
        w2_s = bl16[:, 128:256]
        aw1_s = bl16[:, 256:1024].rearrange("p (k m) -> p k m", m=256)
        aw2_s = bl16[:, 1024:1536].rearrange("p (k m) -> p k m", m=256)
        aw3_s = bl16[:, 1536:1538]

        candT = const.tile([128, ACT_N], F16, tag="candT")
        sums = const.tile([128, G], F32, tag="sums")
        maxs = const.tile([128, G], F32, tag="maxs")
        mm = const.tile([128, 2 * G], F16, tag="mm")       # fp16 pooled (actor)
        pp = const.tile([128, 2, G], F32, tag="pp")        # actor pooled part
        a1 = const.tile([128, 2, ACT_N], F16, tag="a1")
        a2 = const.tile([128, 2, ACT_N], F16, tag="a2")

        zconst = const.tile([128, 1], F16, tag="zconst")
        nc.vector.memset(zconst[:], 0.0)

        def evict(src_ap, dst, engine, bias, gcol=None, gsum=None):
            # out = relu(src + bias) (cast to dst dtype); accum += sum(out)
            acc = None if gcol is None else gsum[:, gcol:gcol + 1]
            if engine == "dve":
                zb = zconst[:, 0:1].broadcast_to(list(src_ap.shape))
                nc.vector.scalar_tensor_tensor(
                    out=dst, in0=src_ap, scalar=bias, in1=zb,
                    op0=ADD, op1=MAX, accum_out=acc)
            else:
                nc.scalar.activation(dst, src_ap, RELU, bias=bias, accum_out=acc)

        def actor_chunk(c):
            # actor MLP + masked softmax for graphs 2c, 2c+1
            lo, hi = c * CHUNK, (c + 1) * CHUNK
            gg = 2 * c
            nc.vector.tensor_scalar(out=mm[:, gg:gg + 2], in0=sums[:, gg:gg + 2],
                                    scalar1=1.0 / N_NODES, scalar2=None, op0=MULT)
            nc.vector.tensor_copy(mm[:, G + gg:G + gg + 2], maxs[:, gg:gg + 2])
            for m in range(2):
                ps = pspool.tile([128, 2048], F32, tag="ps")
                nc.tensor.matmul(ps[:, :2], aw1_s[:, 1, m * 128:(m + 1) * 128],
                                 mm[:, gg:gg + 2], start=True, stop=False)
                nc.tensor.matmul(ps[:, :2], aw1_s[:, 2, m * 128:(m + 1) * 128],
                                 mm[:, G + gg:G + gg + 2], start=False, stop=True)
                nc.vector.tensor_copy(pp[:, m, gg:gg + 2], ps[:, :2])
            for m in range(2):
                ps = pspool.tile([128, 2048], F32, tag="ps")
                nc.tensor.matmul(ps[:, :CHUNK],
                                 aw1_s[:, 0, m * 128:(m + 1) * 128], candT[:, lo:hi])
                tmp = tpool.tile([128, CHUNK], F32, tag="a1tmp")
                ppb = pp[:, m, gg:gg + 2].unsqueeze(2).broadcast_to([128, 2, CAND])
                nc.vector.tensor_tensor(tmp[:], ps[:, :CHUNK], ppb, op=ADD)
                nc.scalar.activation(a1[:, m, lo:hi], tmp[:], TANH, bias=ab1_s[:, m:m + 1])
            for m in range(2):
                ps = pspool.tile([128, 2048], F32, tag="ps")
                nc.tensor.matmul(ps[:, :CHUNK],
                                 aw2_s[:, 0, m * 128:(m + 1) * 128], a1[:, 0, lo:hi],
                                 start=True, stop=False)
                nc.tensor.matmul(ps[:, :CHUNK],
                                 aw2_s[:, 1, m * 128:(m + 1) * 128], a1[:, 1, lo:hi],
                                 start=False, stop=True)
                nc.scalar.activation(a2[:, m, lo:hi], ps[:, :CHUNK], TANH,
                                     bias=ab2_s[:, m:m + 1])
            sps = pspool.tile([128, 2048], F32, tag="ps")
            nc.tensor.matmul(sps[0:1, :CHUNK], aw3_s[:, 0:1], a2[:, 0, lo:hi],
                             start=True, stop=False)
            nc.tensor.matmul(sps[0:1, :CHUNK], aw3_s[:, 1:2], a2[:, 1, lo:hi],
                             start=False, stop=True)
            sm = tpool.tile([1, CHUNK], F32, tag="sm")
            nc.vector.tensor_tensor(sm[:], sps[0:1, :CHUNK], msk_s[0:1, lo:hi], op=ADD)
            es = tpool.tile([1, CHUNK], F32, tag="es")
            nc.scalar.activation(es[:], sm[:], EXP)
            ssum = tpool.tile([1, 2], F32, tag="ssum")
            nc.vector.tensor_reduce(ssum[:], es[:].rearrange("p (g c) -> p g c", g=2),
                                    axis=AXX, op=ADD)
            rec = tpool.tile([1, 2], F32, tag="rec")
            nc.vector.reciprocal(rec[:], ssum[:])
            pi = tpool.tile([1, CHUNK], F32, tag="pi")
            recb = rec[:].unsqueeze(2).broadcast_to([1, 2, CAND])
            nc.vector.tensor_tensor(pi[:].rearrange("p (g c) -> p g c", g=2),
                                    es[:].rearrange("p (g c) -> p g c", g=2),
                                    recb, op=MULT)
            nc.sync.dma_start(out_pi[0:1, lo:hi], pi[:])

        # ---- per-graph main loop (actor chunks interleaved after odd graphs)
        for g in range(G):
            xg = xpool.tile([128, NH // 4], F16, tag="x")
            nc.sync.dma_start(xg[:], xp[g])
            idxg = gpool.tile([128, CAND // 16], U16, tag="idx")
            nc.sync.dma_start(idxg[:], cidx[g])

            # L1: 4 concurrent row-tiled matmuls per wall fill a [128,2048]
            # PSUM chunk; evict (split ACT/DVE) fuses bias+relu+cast to fp16.
            hg = hpool.tile([128, NH], F16, tag="h")
            for w, (qoff, n) in enumerate(L1_WALLS):
                ps = pspool.tile([128, 2048], F32, tag="ps")
                for q in range(4):
                    tp = (96, 0) if q == 3 else None
                    nc.tensor.matmul(
                        ps[:, 512 * q:512 * q + n],
                        w1_s[32 * q:32 * q + 6, :],
                        xg[32 * q:32 * q + 6, qoff:qoff + n],
                        tile_position=tp,
                    )
                base = L1_CHUNK_BASE[w]
                l1eng = "dve" if g == 0 else "act"
                if n == 512:
                    evict(ps[:, 0:2048], hg[:, base:base + 2048], l1eng,
                          b1_s[:, 0:1])
                else:
                    ps3 = ps[:, :].rearrange("p (b c) -> p b c", c=512)[:, :, 0:n]
                    evict(ps3, hg[:, base:base + 4 * n], l1eng, b1_s[:, 0:1])

            # L2: interleaved even/odd row-group matmuls fill [128,2048]
            # chunks; evict (split ACT/DVE) fuses bias+relu+cast+mean-accum.
            rg = rpool.tile([128, 2 * NH], F16, tag="r")
            gsum = gpool.tile([128, 6], F32, tag="gsum")
            da = gpool.tile([128, 2500], F16, tag="da")
            db = gpool.tile([128, 2500], F16, tag="db")
            for t in range(5):
                ps = pspool.tile([128, 2048], F32, tag="ps")
                for k in range(2):
                    m = 2 * t + k
                    hoff, n = L2_NT[m]
                    for s in range(2):
                        p0 = 64 * s
                        nc.tensor.matmul(
                            ps[:, 512 * (2 * k + s):512 * (2 * k + s) + n],
                            w2_s[p0:p0 + 64, :],
                            hg[p0:p0 + 64, hoff:hoff + n],
                        )
                if t < 4:
                    eng = "dve" if t in (2, 3) else "act"
                    evict(ps[:, 0:2048], rg[:, 2048 * t:2048 * (t + 1)], eng,
                          b2_s[:, 0:1], t, gsum)
                    if t == 2:
                        # first max fold needs only cols 0-5000 (chunks 0-2)
                        nc.vector.tensor_tensor(da[:], rg[:, 0:2500],
                                                rg[:, 2500:5000], op=MAX)
                else:
                    ps3 = ps[:, :].rearrange("p (b c) -> p b c", c=512)
                    evict(ps[:, 0:1024], rg[:, 8192:9216], "act",
                          b2_s[:, 0:1], 4, gsum)
                    evict(ps3[:, 2:4, 0:392], rg[:, 9216:10000], "dve",
                          b2_s[:, 0:1], 5, gsum)

            # candidate gather
            nc.gpsimd.indirect_copy(
                candT[:, g * CAND:(g + 1) * CAND],
                rg[:],
                idxg[:],
                i_know_ap_gather_is_preferred=True,
            )

            # mean-pool: total per-graph sum (scaled later)
            nc.vector.tensor_reduce(sums[:, g:g + 1], gsum[:], axis=AXX, op=ADD)

            # max-pool: rest of the pairwise TT-max tree on DVE (fp16 2x)
            nc.vector.tensor_tensor(db[:], rg[:, 5000:7500], rg[:, 7500:10000], op=MAX)
            nc.vector.tensor_tensor(da[:], da[:], db[:], op=MAX)
            nc.vector.tensor_tensor(da[:, 0:1250], da[:, 0:1250], da[:, 1250:2500], op=MAX)
            nc.vector.tensor_tensor(da[:, 0:625], da[:, 0:625], da[:, 625:1250], op=MAX)
            nc.vector.tensor_reduce(maxs[:, g:g + 1], da[:, 0:625], axis=AXX, op=MAX)


        # ---- critic (tiny, fp32): v = cw3^T tanh(cw2^T tanh(cw1^T pooled))
        meanf = const.tile([128, G], F32, tag="meanf")
        nc.vector.tensor_scalar(out=meanf[:], in0=sums[:], scalar1=1.0 / N_NODES,
                                scalar2=None, op0=MULT)
        c1 = const.tile([128, 2, G], F32, tag="c1")
        for m in range(2):
            ps = pspool.tile([128, 2048], F32, tag="ps")
            nc.tensor.matmul(ps[:, :G], cw1_s[:, 0, m * 128:(m + 1) * 128],
                             meanf[:], start=True, stop=False)
            nc.tensor.matmul(ps[:, :G], cw1_s[:, 1, m * 128:(m + 1) * 128],
                             maxs[:], start=False, stop=True)
            nc.scalar.activation(c1[:, m, :], ps[:, :G], TANH, bias=cb1_s[:, m:m + 1])
        c2 = const.tile([128, 2, G], F32, tag="c2")
        for m in range(2):
            ps = pspool.tile([128, 2048], F32, tag="ps")
            nc.tensor.matmul(ps[:, :G], cw2_s[:, 0, m * 128:(m + 1) * 128],
                             c1[:, 0, :], start=True, stop=False)
            nc.tensor.matmul(ps[:, :G], cw2_s[:, 1, m * 128:(m + 1) * 128],
                             c1[:, 1, :], start=False, stop=True)
            nc.scalar.activation(c2[:, m, :], ps[:, :G], TANH, bias=cb2_s[:, m:m + 1])
        vps = pspool.tile([128, 2048], F32, tag="ps")
        nc.tensor.matmul(vps[0:1, :G], cw3_s[:, 0:1], c2[:, 0, :],
                         start=True, stop=False)
        nc.tensor.matmul(vps[0:1, :G], cw3_s[:, 1:2], c2[:, 1, :],
                         start=False, stop=True)
        vsb = tpool.tile([1, G], F32, tag="v")
        nc.vector.tensor_scalar(out=vsb[:], in0=vps[0:1, :G],
                                scalar1=cb3_s[0:1, 0:1], scalar2=None, op0=ADD)
        nc.sync.dma_start(out_v[:], vsb[:])

        # ---- actor chunks (after the main loop to keep PSUM free for it)
        for c in range(4):
            actor_chunk(c)

    nc.compile()
    return nc


# ---------------------------------------------------------------- host packing
def _f16(a):
    return np.asarray(a, np.float32).astype(np.float16)


def _pack_shared(i):
    """Weights / biases shared by all cores, packed into two SBUF-layout blobs."""
    f32 = np.float32
    fe_w1 = np.asarray(i["fe_w1"], f32)
    fe_b1 = np.asarray(i["fe_b1"], f32)
    fe_w2 = np.asarray(i["fe_w2"], f32)
    fe_b2 = np.asarray(i["fe_b2"], f32)

    w1bd = np.zeros((128, 128), f32)
    for q in range(4):
        w1bd[32 * q + 0:32 * q + 3, 0:64] = fe_w1
        w1bd[32 * q + 3:32 * q + 6, 64:128] = fe_w1
    w2ab = np.concatenate([fe_w2, fe_w2], axis=0)          # [128,128]

    def cols2(v):                                          # [256] -> [128, 2]
        return np.asarray(v, f32).reshape(2, 128).T

    def kpm(w, kk):                                        # [kk*128, m] -> [128, kk*m]
        w = np.asarray(w, f32)
        m = w.shape[1]
        return w.reshape(kk, 128, m).transpose(1, 0, 2).reshape(128, kk * m)

    blob32 = np.zeros((128, 1037), f32)
    blob32[:, 0] = np.concatenate([fe_b1, fe_b1])
    blob32[:, 1] = fe_b2
    blob32[:, 2:4] = cols2(i["a_b1"])
    blob32[:, 4:6] = cols2(i["a_b2"])
    blob32[:, 6:8] = cols2(i["c_b1"])
    blob32[:, 8:10] = cols2(i["c_b2"])
    blob32[0, 10] = np.asarray(i["c_b3"], f32).reshape(-1)[0]
    blob32[:, 11:523] = kpm(i["c_w1"], 2)
    blob32[:, 523:1035] = kpm(i["c_w2"], 2)
    blob32[:, 1035:1037] = np.asarray(i["c_w3"], f32).reshape(2, 128).T

    blob16 = np.zeros((128, 1538), np.float16)
    blob16[:, 0:128] = w1bd
    blob16[:, 128:256] = w2ab
    blob16[:, 256:1024] = kpm(i["a_w1"], 3)
    blob16[:, 1024:1536] = kpm(i["a_w2"], 2)
    blob16[:, 1536:1538] = np.asarray(i["a_w3"], f32).reshape(2, 128).T

    return {"blob32": blob32, "blob16": blob16}


def _pack_core(x, candidate, mask, core):
    gs = slice(core * G, (core + 1) * G)
    xg = np.asarray(x[gs], np.float32)                     # [8,10000,3]
    # quarter-packed: partition 32q+r (r<6), col c -> x[g, 2500q + 2c + r//3, r%3]
    xq = (xg.reshape(G, 4, NH // 4, 2, 3)                  # g q c par dim
          .transpose(0, 1, 3, 4, 2)                        # g q par dim c
          .reshape(G, 4, 6, NH // 4))
    xp = np.zeros((G, 4, 32, NH // 4), np.float16)
    xp[:, :, 0:6, :] = xq
    xp = np.ascontiguousarray(xp.reshape(G, 128, NH // 4))

    cand = np.asarray(candidate[gs]).astype(np.int64)      # [8,201]
    # node -> r column under the L1 wall layout + L2 interleaved chunks
    q = cand // 2500
    i = cand - 2500 * q
    j = i // 2
    s = i % 2
    hc = np.where(j < 512, 512 * q + j,
         np.where(j < 1024, 2048 + 512 * q + (j - 512),
                  4096 + 226 * q + (j - 1024)))
    m = hc // 512
    c = hc - 512 * m
    rb = np.array([[_r_col_base(st, mt) for mt in range(10)] for st in range(2)],
                  np.int64)
    cand_r = rb[s, m] + c
    cp = np.zeros((G, CAND), np.int64)
    cp[:, :N_CAND] = cand_r
    # wrap for ap_gather: index j lives at [16*grp + j%16, j//16] for all groups
    idxw = np.zeros((G, 16, CAND // 16), np.uint16)
    j = np.arange(CAND)
    idxw[:, j % 16, j // 16] = cp.astype(np.uint16)
    cidx = np.ascontiguousarray(np.tile(idxw, (1, 8, 1)))  # [8,128,13]

    m = np.asarray(mask[gs]).astype(bool)                  # [8,201]
    mp = np.full((G, CAND), BIG_NEG, np.float32)
    mp[:, :N_CAND] = np.where(m, 0.0, BIG_NEG)
    mskp = np.ascontiguousarray(mp.reshape(1, ACT_N))

    return {"xp": xp, "cidx": cidx, "mskp": mskp}


_CACHE = {}


def _get_nc():
    if "nc" not in _CACHE:
        _CACHE["nc"] = _build_bass()
    return _CACHE["nc"]


def _run(inputs, trace=False):
    nc = _get_nc()
    shared = _pack_shared(inputs)
    x, candidate, mask = inputs["x"], inputs["candidate"], inputs["mask"]
    in_maps = []
    for core in range(NCORES):
        m = dict(shared)
        m.update(_pack_core(x, candidate, mask, core))
        in_maps.append(m)
    res = run_bass_kernel_spmd(nc, in_maps, core_ids=list(range(NCORES)),
                               trace=trace)
    pi = np.concatenate(
        [r["out_pi"].reshape(G, CAND)[:, :N_CAND] for r in res.results], axis=0)
    v = np.concatenate([r["out_v"].reshape(G) for r in res.results], axis=0)
    return (pi.reshape(B, N_CAND, 1).astype(np.float32),
            v.reshape(B, 1).astype(np.float32)), res


def kernel(**inputs):
    out, _ = _run(inputs, trace=False)
    return out


# revision 32
# speedup vs baseline: 1.5755x; 1.2173x over previous
"""Trainium2 Bass kernel for an ActorCritic module (JSSP-style).

Math (per graph b of 64):
  h       = relu(x @ fe_w1 + fe_b1)                  x:[10000,3] -> h:[10000,64]
  h_nodes = relu(h @ fe_w2 + fe_b2)                  -> [10000,128]
  pooled  = [mean_n h_nodes, max_n h_nodes]          -> [256]
  cand    = h_nodes[candidate]                       -> [201,128]
  scores  = tanh-MLP([cand, pooled]) -> softmax(masked) -> pi [201,1]
  v       = tanh-MLP(pooled)                         -> [1]

Sharding: pure data-parallel over batch, 8 graphs per NeuronCore, all weights
replicated, no collectives. Outputs gathered on the host.

Device design (per core, feature-on-partition layout, fp16 compute with fp32
PSUM accumulation; critic in fp32):
 - x is host-packed two-nodes-per-column and quartered so layer 1 runs as four
   concurrent row-tiled matmuls (tile_position row groups, K=6 each, M=128).
 - Layer 2 runs as two concurrent row-group matmuls (even nodes' features on
   partitions 0-63, odd on 64-127), interleaved into [128,2048] PSUM chunks.
 - PSUM eviction fuses bias + relu + fp16 cast + the mean-pool accumulation
   (ScalarE activation accum_out; DVE scalar_tensor_tensor accum for the
   chunks assigned to DVE for engine balance).
 - Max-pool is a pairwise tensor_tensor max tree on DVE (fp16 2x mode).
 - Candidate gather via GPSIMD indirect_copy with host-precomputed wrapped
   uint16 column indices (the node->column remap is done on the host).
 - Actor MLP batched over 2-graph chunks; softmax uses exp without max
   subtraction (scores are tiny; masked entries get -30000 -> exp == 0).
 - All weights/biases ship as two pre-packed SBUF-layout blobs (one f32, one
   f16) so startup is three DMAs instead of seventeen.

Measured on trn2 (8 cores): ~164 us exec, pi relerr ~5e-7, v relerr ~1.3e-3.
"""

import os
from contextlib import ExitStack

import numpy as np

import concourse.bass as bass
from concourse import bacc
import concourse.mybir as mybir
import concourse.tile as tile
from concourse.bass_utils import run_bass_kernel_spmd

# ---------------------------------------------------------------- constants
B, N_NODES, IN_DIM = 64, 10000, 3
N_CAND = 201
HID, ND, HA = 64, 128, 256       # hidden, node dim, actor/critic hidden
NCORES = 8
G = B // NCORES                  # graphs per core = 8
NH = N_NODES // 2                # packed columns per graph = 5000
CAND = 208                       # candidates padded to a multiple of 16
ACT_N = G * CAND                 # actor batch columns = 1664
CHUNK = 2 * CAND                 # actor n-chunk (2 graphs) = 416
BIG_NEG = -30000.0

F32 = mybir.dt.float32
F16 = mybir.dt.float16
U16 = mybir.dt.uint16

ADD = mybir.AluOpType.add
MAX = mybir.AluOpType.max
MULT = mybir.AluOpType.mult
RELU = mybir.ActivationFunctionType.Relu
TANH = mybir.ActivationFunctionType.Tanh
EXP = mybir.ActivationFunctionType.Exp
AXX = mybir.AxisListType.X


def _col_tiles(total, step=512):
    tiles, off = [], 0
    while off < total:
        n = min(step, total - off)
        tiles.append((off, n))
        off += n
    return tiles


L2_NT = _col_tiles(NH)           # 10 n-tiles per stream: 9x512 + 392

# L1 is 4-way row-tiled: quarter q covers nodes [2500q, 2500q+2500), packed
# two nodes per column (1250 quarter-columns). PSUM fill walls per graph:
L1_WALLS = [(0, 512), (512, 512), (1024, 226)]     # (quarter-col offset, n)
L1_CHUNK_BASE = [0, 2048, 4096]                    # h-col base per wall chunk


def _h_col(q, j):
    # h column of quarter q, quarter-column j under the wall layout
    if j < 512:
        return 512 * q + j
    if j < 1024:
        return 2048 + 512 * q + (j - 512)
    return 4096 + 226 * q + (j - 1024)


def _r_col_base(s, m):
    # r column base of L2 stream s (0=even node, 1=odd), n-tile m (0..9)
    t, k = divmod(m, 2)
    sizes = [L2_NT[2 * t][1], L2_NT[2 * t][1],
             L2_NT[2 * t + 1][1], L2_NT[2 * t + 1][1]]
    idx = 2 * k + s
    return 2048 * t + sum(sizes[:idx])


# Which eviction chunks run on DVE (rest on ScalarE). Tuned for engine balance.
L1_DVE = {1}
L2_DVE = {2}


# ---------------------------------------------------------------- builder
DVE_W = 512        # columns of each clean 2048-chunk evicted by DVE (rest ACT)


def _build_bass():
    nc = bacc.Bacc("TRN2", debug=False)

    xp = nc.declare_dram_parameter("xp", [G, 128, NH // 4], F16, isOutput=False)
    # fp32 blob cols: b1(1) b2(1) ab1(2) ab2(2) cb1(2) cb2(2) cb3(1) cw1(512) cw2(512) cw3(2)
    blob32 = nc.declare_dram_parameter("blob32", [128, 1037], F32, isOutput=False)
    # fp16 blob cols: w1(128) w2(128) aw1(768) aw2(512) aw3(2)
    blob16 = nc.declare_dram_parameter("blob16", [128, 1538], F16, isOutput=False)
    cidx = nc.declare_dram_parameter("cidx", [G, 128, CAND // 16], U16, isOutput=False)
    mskp = nc.declare_dram_parameter("mskp", [1, ACT_N], F32, isOutput=False)
    out_pi = nc.declare_dram_parameter("out_pi", [1, ACT_N], F32, isOutput=True)
    out_v = nc.declare_dram_parameter("out_v", [1, G], F32, isOutput=True)

    with tile.TileContext(nc) as tc, ExitStack() as ctx:
        const = ctx.enter_context(tc.tile_pool(name="const", bufs=1))
        xpool = ctx.enter_context(tc.tile_pool(name="xp", bufs=3))
        hpool = ctx.enter_context(tc.tile_pool(name="hp", bufs=2))
        rpool = ctx.enter_context(tc.tile_pool(name="rp", bufs=3))
        gpool = ctx.enter_context(tc.tile_pool(name="gp", bufs=3))
        tpool = ctx.enter_context(tc.tile_pool(name="tp", bufs=2))
        pspool = ctx.enter_context(tc.tile_pool(name="ps", bufs=2, space="PSUM"))

        # ---- load weights / constants
        def load(shape, dtype, src, name):
            t = const.tile(shape, dtype, tag=name)
            nc.sync.dma_start(t[:], src)
            return t

        bl32 = load([128, 1037], F32, blob32[:], "bl32")
        bl16 = load([128, 1538], F16, blob16[:], "bl16")
        msk_s = load([1, ACT_N], F32, mskp[:], "msk")
        b1_s = bl32[:, 0:1]
        b2_s = bl32[:, 1:2]
        ab1_s = bl32[:, 2:4]
        ab2_s = bl32[:, 4:6]
        cb1_s = bl32[:, 6:8]
        cb2_s = bl32[:, 8:10]
        cb3_s = bl32[:, 10:11]
        cw1_s = bl32[:, 11:523].rearrange("p (k m) -> p k m", m=256)
        cw2_s = bl32[:, 523:1035].rearrange("p (k m) -> p k m", m=256)
        cw3_s = bl32[:, 1035:1037]
        w1_s = bl16[:, 0:128]
        w2_s = bl16[:, 128:256]
        aw1_s = bl16[:, 256:1024].rearrange("p (k m) -> p k m", m=256)
        aw2_s = bl16[:, 1024:1536].rearrange("p (k m) -> p k m", m=256)
        aw3_s = bl16[:, 1536:1538]

        candT = const.tile([128, ACT_N], F16, tag="candT")
        sums = const.tile([128, G], F32, tag="sums")
        maxs = const.tile([128, G], F32, tag="maxs")
        mm = const.tile([128, 2 * G], F16, tag="mm")       # fp16 pooled (actor)
        pp = const.tile([128, 2, G], F32, tag="pp")        # actor pooled part
        a1 = const.tile([128, 2, ACT_N], F16, tag="a1")
        a2 = const.tile([128, 2, ACT_N], F16, tag="a2")

        zconst = const.tile([128, 1], F16, tag="zconst")
        nc.vector.memset(zconst[:], 0.0)

        def evict(src_ap, dst, engine, bias, gcol=None, gsum=None):
            # out = relu(src + bias) (cast to dst dtype); accum += sum(out)
            acc = None if gcol is None else gsum[:, gcol:gcol + 1]
            if engine == "dve":
                zb = zconst[:, 0:1].broadcast_to(list(src_ap.shape))
                nc.vector.scalar_tensor_tensor(
                    out=dst, in0=src_ap, scalar=bias, in1=zb,
                    op0=ADD, op1=MAX, accum_out=acc)
            else:
                nc.scalar.activation(dst, src_ap, RELU, bias=bias, accum_out=acc)

        def actor_chunk(c):
            # actor MLP + masked softmax for graphs 2c, 2c+1
            lo, hi = c * CHUNK, (c + 1) * CHUNK
            gg = 2 * c
            nc.vector.tensor_scalar(out=mm[:, gg:gg + 2], in0=sums[:, gg:gg + 2],
                                    scalar1=1.0 / N_NODES, scalar2=None, op0=MULT)
            nc.vector.tensor_copy(mm[:, G + gg:G + gg + 2], maxs[:, gg:gg + 2])
            for m in range(2):
                ps = pspool.tile([128, 2048], F32, tag="ps")
                nc.tensor.matmul(ps[:, :2], aw1_s[:, 1, m * 128:(m + 1) * 128],
                                 mm[:, gg:gg + 2], start=True, stop=False)
                nc.tensor.matmul(ps[:, :2], aw1_s[:, 2, m * 128:(m + 1) * 128],
                                 mm[:, G + gg:G + gg + 2], start=False, stop=True)
                nc.vector.tensor_copy(pp[:, m, gg:gg + 2], ps[:, :2])
            for m in range(2):
                ps = pspool.tile([128, 2048], F32, tag="ps")
                nc.tensor.matmul(ps[:, :CHUNK],
                                 aw1_s[:, 0, m * 128:(m + 1) * 128], candT[:, lo:hi])
                tmp = tpool.tile([128, CHUNK], F32, tag="a1tmp")
                ppb = pp[:, m, gg:gg + 2].unsqueeze(2).broadcast_to([128, 2, CAND])
                nc.vector.tensor_tensor(tmp[:], ps[:, :CHUNK], ppb, op=ADD)
                nc.scalar.activation(a1[:, m, lo:hi], tmp[:], TANH, bias=ab1_s[:, m:m + 1])
            for m in range(2):
                ps = pspool.tile([128, 2048], F32, tag="ps")
                nc.tensor.matmul(ps[:, :CHUNK],
                                 aw2_s[:, 0, m * 128:(m + 1) * 128], a1[:, 0, lo:hi],
                                 start=True, stop=False)
                nc.tensor.matmul(ps[:, :CHUNK],
                                 aw2_s[:, 1, m * 128:(m + 1) * 128], a1[:, 1, lo:hi],
                                 start=False, stop=True)
                nc.scalar.activation(a2[:, m, lo:hi], ps[:, :CHUNK], TANH,
                                     bias=ab2_s[:, m:m + 1])
            sps = pspool.tile([128, 2048], F32, tag="ps")
            nc.tensor.matmul(sps[0:1, :CHUNK], aw3_s[:, 0:1], a2[:, 0, lo:hi],
                             start=True, stop=False)
            nc.tensor.matmul(sps[0:1, :CHUNK], aw3_s[:, 1:2], a2[:, 1, lo:hi],
                             start=False, stop=True)
            sm = tpool.tile([1, CHUNK], F32, tag="sm")
            nc.vector.tensor_tensor(sm[:], sps[0:1, :CHUNK], msk_s[0:1, lo:hi], op=ADD)
            es = tpool.tile([1, CHUNK], F32, tag="es")
            nc.scalar.activation(es[:], sm[:], EXP)
            ssum = tpool.tile([1, 2], F32, tag="ssum")
            nc.vector.tensor_reduce(ssum[:], es[:].rearrange("p (g c) -> p g c", g=2),
                                    axis=AXX, op=ADD)
            rec = tpool.tile([1, 2], F32, tag="rec")
            nc.vector.reciprocal(rec[:], ssum[:])
            pi = tpool.tile([1, CHUNK], F32, tag="pi")
            recb = rec[:].unsqueeze(2).broadcast_to([1, 2, CAND])
            nc.vector.tensor_tensor(pi[:].rearrange("p (g c) -> p g c", g=2),
                                    es[:].rearrange("p (g c) -> p g c", g=2),
                                    recb, op=MULT)
            nc.sync.dma_start(out_pi[0:1, lo:hi], pi[:])

        # ---- per-graph main loop (actor chunks interleaved after odd graphs)
        for g in range(G):
            xg = xpool.tile([128, NH // 4], F16, tag="x")
            nc.sync.dma_start(xg[:], xp[g])
            idxg = gpool.tile([128, CAND // 16], U16, tag="idx")
            nc.sync.dma_start(idxg[:], cidx[g])

            # L1: 4 concurrent row-tiled matmuls per wall fill a [128,2048]
            # PSUM chunk; evict (split ACT/DVE) fuses bias+relu+cast to fp16.
            hg = hpool.tile([128, NH], F16, tag="h")
            for w, (qoff, n) in enumerate(L1_WALLS):
                ps = pspool.tile([128, 2048], F32, tag="ps")
                for q in range(4):
                    tp = (96, 0) if q == 3 else None
                    nc.tensor.matmul(
                        ps[:, 512 * q:512 * q + n],
                        w1_s[32 * q:32 * q + 6, :],
                        xg[32 * q:32 * q + 6, qoff:qoff + n],
                        tile_position=tp,
                    )
                base = L1_CHUNK_BASE[w]
                if n == 512:
                    evict(ps[:, 0:2048], hg[:, base:base + 2048], "act",
                          b1_s[:, 0:1])
                else:
                    ps3 = ps[:, :].rearrange("p (b c) -> p b c", c=512)[:, :, 0:n]
                    evict(ps3, hg[:, base:base + 4 * n], "act", b1_s[:, 0:1])

            # L2: interleaved even/odd row-group matmuls fill [128,2048]
            # chunks; evict (split ACT/DVE) fuses bias+relu+cast+mean-accum.
            rg = rpool.tile([128, 2 * NH], F16, tag="r")
            gsum = gpool.tile([128, 6], F32, tag="gsum")
            for t in range(5):
                ps = pspool.tile([128, 2048], F32, tag="ps")
                for k in range(2):
                    m = 2 * t + k
                    hoff, n = L2_NT[m]
                    for s in range(2):
                        p0 = 64 * s
                        nc.tensor.matmul(
                            ps[:, 512 * (2 * k + s):512 * (2 * k + s) + n],
                            w2_s[p0:p0 + 64, :],
                            hg[p0:p0 + 64, hoff:hoff + n],
                        )
                if t < 4:
                    eng = "dve" if t in (2, 3) else "act"
                    evict(ps[:, 0:2048], rg[:, 2048 * t:2048 * (t + 1)], eng,
                          b2_s[:, 0:1], t, gsum)
                else:
                    ps3 = ps[:, :].rearrange("p (b c) -> p b c", c=512)
                    evict(ps[:, 0:1024], rg[:, 8192:9216], "act",
                          b2_s[:, 0:1], 4, gsum)
                    evict(ps3[:, 2:4, 0:392], rg[:, 9216:10000], "dve",
                          b2_s[:, 0:1], 5, gsum)

            # candidate gather
            nc.gpsimd.indirect_copy(
                candT[:, g * CAND:(g + 1) * CAND],
                rg[:],
                idxg[:],
                i_know_ap_gather_is_preferred=True,
            )

            # mean-pool: total per-graph sum (scaled later)
            nc.vector.tensor_reduce(sums[:, g:g + 1], gsum[:], axis=AXX, op=ADD)

            # max-pool: pairwise TT-max tree on DVE (fp16 2x mode)
            da = gpool.tile([128, 2500], F16, tag="da")
            db = gpool.tile([128, 2500], F16, tag="db")
            nc.vector.tensor_tensor(da[:], rg[:, 0:2500], rg[:, 2500:5000], op=MAX)
            nc.vector.tensor_tensor(db[:], rg[:, 5000:7500], rg[:, 7500:10000], op=MAX)
            nc.vector.tensor_tensor(da[:], da[:], db[:], op=MAX)
            nc.vector.tensor_tensor(da[:, 0:1250], da[:, 0:1250], da[:, 1250:2500], op=MAX)
            nc.vector.tensor_tensor(da[:, 0:625], da[:, 0:625], da[:, 625:1250], op=MAX)
            nc.vector.tensor_reduce(maxs[:, g:g + 1], da[:, 0:625], axis=AXX, op=MAX)


        # ---- actor chunks (after the main loop to keep PSUM free for it)
        for c in range(4):
            actor_chunk(c)

        # ---- critic (tiny, fp32): v = cw3^T tanh(cw2^T tanh(cw1^T pooled))
        meanf = const.tile([128, G], F32, tag="meanf")
        nc.vector.tensor_scalar(out=meanf[:], in0=sums[:], scalar1=1.0 / N_NODES,
                                scalar2=None, op0=MULT)
        c1 = const.tile([128, 2, G], F32, tag="c1")
        for m in range(2):
            ps = pspool.tile([128, 2048], F32, tag="ps")
            nc.tensor.matmul(ps[:, :G], cw1_s[:, 0, m * 128:(m + 1) * 128],
                             meanf[:], start=True, stop=False)
            nc.tensor.matmul(ps[:, :G], cw1_s[:, 1, m * 128:(m + 1) * 128],
                             maxs[:], start=False, stop=True)
            nc.scalar.activation(c1[:, m, :], ps[:, :G], TANH, bias=cb1_s[:, m:m + 1])
        c2 = const.tile([128, 2, G], F32, tag="c2")
        for m in range(2):
            ps = pspool.tile([128, 2048], F32, tag="ps")
            nc.tensor.matmul(ps[:, :G], cw2_s[:, 0, m * 128:(m + 1) * 128],
                             c1[:, 0, :], start=True, stop=False)
            nc.tensor.matmul(ps[:, :G], cw2_s[:, 1, m * 128:(m + 1) * 128],
                             c1[:, 1, :], start=False, stop=True)
            nc.scalar.activation(c2[:, m, :], ps[:, :G], TANH, bias=cb2_s[:, m:m + 1])
        vps = pspool.tile([128, 2048], F32, tag="ps")
        nc.tensor.matmul(vps[0:1, :G], cw3_s[:, 0:1], c2[:, 0, :],
                         start=True, stop=False)
        nc.tensor.matmul(vps[0:1, :G], cw3_s[:, 1:2], c2[:, 1, :],
                         start=False, stop=True)
        vsb = tpool.tile([1, G], F32, tag="v")
        nc.vector.tensor_scalar(out=vsb[:], in0=vps[0:1, :G],
                                scalar1=cb3_s[0:1, 0:1], scalar2=None, op0=ADD)
        nc.sync.dma_start(out_v[:], vsb[:])

    nc.compile()
    return nc


# ---------------------------------------------------------------- host packing
def _f16(a):
    return np.asarray(a, np.float32).astype(np.float16)


def _pack_shared(i):
    """Weights / biases shared by all cores, packed into two SBUF-layout blobs."""
    f32 = np.float32
    fe_w1 = np.asarray(i["fe_w1"], f32)
    fe_b1 = np.asarray(i["fe_b1"], f32)
    fe_w2 = np.asarray(i["fe_w2"], f32)
    fe_b2 = np.asarray(i["fe_b2"], f32)

    w1bd = np.zeros((128, 128), f32)
    for q in range(4):
        w1bd[32 * q + 0:32 * q + 3, 0:64] = fe_w1
        w1bd[32 * q + 3:32 * q + 6, 64:128] = fe_w1
    w2ab = np.concatenate([fe_w2, fe_w2], axis=0)          # [128,128]

    def cols2(v):                                          # [256] -> [128, 2]
        return np.asarray(v, f32).reshape(2, 128).T

    def kpm(w, kk):                                        # [kk*128, m] -> [128, kk*m]
        w = np.asarray(w, f32)
        m = w.shape[1]
        return w.reshape(kk, 128, m).transpose(1, 0, 2).reshape(128, kk * m)

    blob32 = np.zeros((128, 1037), f32)
    blob32[:, 0] = np.concatenate([fe_b1, fe_b1])
    blob32[:, 1] = fe_b2
    blob32[:, 2:4] = cols2(i["a_b1"])
    blob32[:, 4:6] = cols2(i["a_b2"])
    blob32[:, 6:8] = cols2(i["c_b1"])
    blob32[:, 8:10] = cols2(i["c_b2"])
    blob32[0, 10] = np.asarray(i["c_b3"], f32).reshape(-1)[0]
    blob32[:, 11:523] = kpm(i["c_w1"], 2)
    blob32[:, 523:1035] = kpm(i["c_w2"], 2)
    blob32[:, 1035:1037] = np.asarray(i["c_w3"], f32).reshape(2, 128).T

    blob16 = np.zeros((128, 1538), np.float16)
    blob16[:, 0:128] = w1bd
    blob16[:, 128:256] = w2ab
    blob16[:, 256:1024] = kpm(i["a_w1"], 3)
    blob16[:, 1024:1536] = kpm(i["a_w2"], 2)
    blob16[:, 1536:1538] = np.asarray(i["a_w3"], f32).reshape(2, 128).T

    return {"blob32": blob32, "blob16": blob16}


def _pack_core(x, candidate, mask, core):
    gs = slice(core * G, (core + 1) * G)
    xg = np.asarray(x[gs], np.float32)                     # [8,10000,3]
    # quarter-packed: partition 32q+r (r<6), col c -> x[g, 2500q + 2c + r//3, r%3]
    xq = (xg.reshape(G, 4, NH // 4, 2, 3)                  # g q c par dim
          .transpose(0, 1, 3, 4, 2)                        # g q par dim c
          .reshape(G, 4, 6, NH // 4))
    xp = np.zeros((G, 4, 32, NH // 4), np.float16)
    xp[:, :, 0:6, :] = xq
    xp = np.ascontiguousarray(xp.reshape(G, 128, NH // 4))

    cand = np.asarray(candidate[gs]).astype(np.int64)      # [8,201]
    # node -> r column under the L1 wall layout + L2 interleaved chunks
    q = cand // 2500
    i = cand - 2500 * q
    j = i // 2
    s = i % 2
    hc = np.where(j < 512, 512 * q + j,
         np.where(j < 1024, 2048 + 512 * q + (j - 512),
                  4096 + 226 * q + (j - 1024)))
    m = hc // 512
    c = hc - 512 * m
    rb = np.array([[_r_col_base(st, mt) for mt in range(10)] for st in range(2)],
                  np.int64)
    cand_r = rb[s, m] + c
    cp = np.zeros((G, CAND), np.int64)
    cp[:, :N_CAND] = cand_r
    # wrap for ap_gather: index j lives at [16*grp + j%16, j//16] for all groups
    idxw = np.zeros((G, 16, CAND // 16), np.uint16)
    j = np.arange(CAND)
    idxw[:, j % 16, j // 16] = cp.astype(np.uint16)
    cidx = np.ascontiguousarray(np.tile(idxw, (1, 8, 1)))  # [8,128,13]

    m = np.asarray(mask[gs]).astype(bool)                  # [8,201]
    mp = np.full((G, CAND), BIG_NEG, np.float32)
    mp[:, :N_CAND] = np.where(m, 0.0, BIG_NEG)
    mskp = np.ascontiguousarray(mp.reshape(1, ACT_N))

    return {"xp": xp, "cidx": cidx, "mskp": mskp}


_CACHE = {}


def _get_nc():
    if "nc" not in _CACHE:
        _CACHE["nc"] = _build_bass()
    return _CACHE["nc"]


def _run(inputs, trace=False):
    nc = _get_nc()
    shared = _pack_shared(inputs)
    x, candidate, mask = inputs["x"], inputs["candidate"], inputs["mask"]
    in_maps = []
    for core in range(NCORES):
        m = dict(shared)
        m.update(_pack_core(x, candidate, mask, core))
        in_maps.append(m)
    res = run_bass_kernel_spmd(nc, in_maps, core_ids=list(range(NCORES)),
                               trace=trace)
    pi = np.concatenate(
        [r["out_pi"].reshape(G, CAND)[:, :N_CAND] for r in res.results], axis=0)
    v = np.concatenate([r["out_v"].reshape(G) for r in res.results], axis=0)
    return (pi.reshape(B, N_CAND, 1).astype(np.float32),
            v.reshape(B, 1).astype(np.float32)), res


def kernel(**inputs):
    out, _ = _run(inputs, trace=False)
    return out
